# revision 24
# baseline (speedup 1.0000x reference)
"""Two-block single-head transformer (B=4, S=4096, E=256) on 8 TRN2 NeuronCores.

Sharding: core c -> batch b=c//2, query-half h=c%2 (2048 query rows each).
Each core receives its batch's x ROLLED so that its own query rows are always
rows [0:2048] -- this keeps the on-device program identical across cores
(pure SPMD, no partition-id branching).  Attention is permutation-invariant
over keys, so layer-1 may use the rolled key order.  Layer-2 keys come from a
pairwise AllGather of the LN1 outputs in canonical order.

Math per layer (matches torch reference):
  q/k/v = x @ W.T + b ; att = softmax((q k^T)/sqrt(S)) ; o = att v
  layernorm over E with gamma/beta.

Datapath: every matmul runs in fp8-e4m3 with MatmulPerfMode.DoubleRow, which
contracts two 128-row K-subtiles per pass (2x bf16 FLOP rate).  Q^T/K^T
tiles are laid out [128, 2, n] with dim1 = the two halves of E, exp'd score
tiles [128, 2, 512] with dim1 = a pair of 128-key chunks -- both exactly the
DoubleRow operand shape.  Scores for a key-chunk PAIR accumulate into one
2-bank PSUM group and are exp'd by a single ACT instruction (amortizing the
~352-cycle ACT fixed overhead), output straight to fp8 for the att@V matmul.
A ones-column appended to V makes the softmax denominator fall out of the
same accumulation.

The scores->exp->attV chain is software-pipelined: scores for pair j+1 are
emitted BEFORE the att@V matmuls of pair j, so the in-order Tensor stream
never sits behind an ACT wait (period = max(tensor, act) per pair instead of
their sum).

Layer-2 projections read the LN1 output transposed.  The transpose DMA only
supports 2-byte dtypes, so the fp8 LN1 output is transposed as packed uint16
PAIRS (bitcast), giving oT[p, r, b] = o[r, 2p+b]; the layer-2 weights are
host-striped with the matching interleaved E-pairing so the DoubleRow
contraction (dim1 = b) still sums over all of E.

PSUM budget: 4 banks att@V accumulators + 2x2-bank double-buffered score
groups = 8.  Projection matmuls borrow score-group tiles from the same pool.

The layer boundary is pipelined: the AllGather is split into one chunk per
512-query block, and the layer-2 transposes/projections are chunked behind
those, so they overlap the remaining layer-1 attention instead of
serializing after it.
"""

import sys

sys.path.insert(0, "/opt/trn_rl_repo")

import numpy as np
import ml_dtypes

import concourse.bass as bass
import concourse.tile as tile
from concourse import bacc, mybir
from concourse import bass_utils

f32 = mybir.dt.float32
bf16 = mybir.dt.bfloat16
f8 = mybir.dt.float8e4
u16 = mybir.dt.uint16
DR = mybir.MatmulPerfMode.DoubleRow

B, S, E = 4, 4096, 256
P = 128
SQ = S // 2          # query rows per core
DC = E // P          # 2 chunks of the head dim
KC = S // P          # 32 key chunks
NP_ = KC // 2        # 16 key-chunk pairs
QB = 512             # query block (matmul moving dim)
NQB = SQ // QB       # 4 query blocks per core
QS = QB // P         # 4 query sub-blocks per block
N_CORES = 8
EPS = 1e-5
SCALE = 1.0 / np.sqrt(np.float32(S))

_COMPILED = None


def _broadcast_ap(vec_ap, parts, n):
    """[n] DRAM vector -> [parts, n] partition-broadcast access pattern."""
    return bass.AP(
        tensor=vec_ap.tensor,
        offset=vec_ap.offset,
        ap=[[0, parts], [1, n]],
    )


def _bc_mid(ap2d, reps):
    """[P, n] SBUF AP -> [P, reps, n] with a stride-0 middle dim."""
    return bass.AP(
        tensor=ap2d.tensor,
        offset=ap2d.offset,
        ap=[list(ap2d.ap[0]), [0, reps], list(ap2d.ap[1])],
    )


def _build():
    nc = bacc.Bacc(
        "TRN2", target_bir_lowering=False, debug=False, num_devices=N_CORES
    )

    # --- kernel I/O (per core) ---
    xT = nc.dram_tensor("xT", [P, DC * S], f8, kind="ExternalInput").ap()
    xq = nc.dram_tensor("xq", [SQ, E], f32, kind="ExternalInput").ap()
    wts = {
        n: nc.dram_tensor(n, [P, DC * E], f8, kind="ExternalInput").ap()
        for n in ["wqt1", "wkt1", "wvt1", "wqt2", "wkt2", "wvt2"]
    }
    vecs = {
        n: nc.dram_tensor(n, [E], f32, kind="ExternalInput").ap()
        for n in ["bq1", "bk1", "bv1", "bq2", "bk2", "bv2",
                  "g1", "be1", "g2", "be2"]
    }
    y = nc.dram_tensor("y", [SQ, E], f32, kind="ExternalOutput").ap()

    with tile.TileContext(nc) as tc:
        _emit(nc, tc, xT, xq, wts, vecs, y)

    nc.compile()
    return nc


def _emit(nc, tc, xT, xq, wts, vecs, y):
    from contextlib import ExitStack

    ctx = ExitStack()
    with ctx:
        const = ctx.enter_context(tc.tile_pool(name="const", bufs=1))
        srcT_pool = ctx.enter_context(tc.tile_pool(name="srcT", bufs=1))
        kt_pool = ctx.enter_context(tc.tile_pool(name="kt", bufs=2))
        v_pool = ctx.enter_context(tc.tile_pool(name="v", bufs=2))
        qt_pool = ctx.enter_context(tc.tile_pool(name="qt", bufs=2))
        o_pool = ctx.enter_context(tc.tile_pool(name="okeep", bufs=16))
        work = ctx.enter_context(tc.tile_pool(name="work", bufs=4))
        expp = ctx.enter_context(tc.tile_pool(name="expp", bufs=4))
        stats = ctx.enter_context(tc.tile_pool(name="stats", bufs=8))
        dram = ctx.enter_context(tc.tile_pool(name="dram", bufs=1, space="DRAM"))
        # score-pair groups (2 banks each, double-buffered); projection
        # matmuls borrow tiles from the same pool
        sc_ps = ctx.enter_context(tc.tile_pool(name="sc_ps", bufs=2, space="PSUM"))
        o_ps = ctx.enter_context(tc.tile_pool(name="o_ps", bufs=4, space="PSUM"))

        # --- constants (sync HWDGE queue, in order of first use) ---
        w_sb, bias_sb, bcast_sb = {}, {}, {}

        def _load_w(n):
            t = const.tile([P, DC, E], f8, tag=f"w_{n}", name=f"w_{n}")
            nc.sync.dma_start(
                out=t[:], in_=wts[n].rearrange("p (dc o) -> p dc o", dc=DC)
            )
            w_sb[n] = t

        def _load_b(n):
            t = const.tile([P, DC], f32, tag=f"b_{n}", name=f"b_{n}")
            nc.sync.dma_start(
                out=t[:], in_=vecs[n].rearrange("(dc p) -> p dc", p=P)
            )
            bias_sb[n] = t

        def _load_bc(n):
            t = const.tile([P, E], f32, tag=f"bc_{n}", name=f"bc_{n}")
            nc.sync.dma_start(out=t[:], in_=_broadcast_ap(vecs[n], P, E))
            bcast_sb[n] = t

        for n in ["wkt1", "wvt1", "wqt1"]:
            _load_w(n)
        _load_b("bk1")
        _load_b("bq1")
        _load_bc("bv1")

        def _load_late_consts():
            for n in ["g1", "be1"]:
                _load_bc(n)
            for n in ["wqt2", "wkt2", "wvt2"]:
                _load_w(n)
            for n in ["bq2", "bk2"]:
                _load_b(n)
            for n in ["bv2", "g2", "be2"]:
                _load_bc(n)

        o_bounce = dram.tile([SQ, E], f8)
        # the layer-boundary exchange ships TRANSPOSED chunks: each block's
        # LN1 rows are transposed locally (oqT), bounced to DRAM, and
        # AllGathered as [128, QB] uint16 tiles -- so the layer-2 key source
        # needs only plain contiguous loads after the collective, never a
        # transpose gated on remote data.
        oT_bounce = [
            dram.tile([P, QB], u16, name=f"otb{i}") for i in range(NQB)
        ]
        o_chunksT = [
            dram.tile([2 * P, QB], u16, name=f"agchunk{i}") for i in range(NQB)
        ]

        # ---- fp8 DoubleRow projections.  src is [P, 2, n] with dim1 the
        # contraction-pair index (E-halves for layer 1, E-pairs for the
        # transposed layer-2 source); w must be striped to match. ----
        def proj_kq(dstT, w8, bk, dst_c, src):
            sc = sc_ps.tile([P, 2, QB], f32, tag="sc", name="pkq")
            for oc in range(DC):
                nc.tensor.matmul(
                    sc[:, oc, :],
                    lhsT=w8[:, :, oc * P:(oc + 1) * P],
                    rhs=src,
                    start=True, stop=True, perf_mode=DR,
                )
            for oc in range(DC):
                nc.vector.tensor_scalar_add(
                    out=dstT[:, oc, dst_c * QB:(dst_c + 1) * QB],
                    in0=sc[:, oc, :],
                    scalar1=bk[:, oc:oc + 1],
                )

        def proj_v(v_sb, wv8, bv_bc, dst_c, src4):
            """src4: list of 4 [P, 2, P] stationary slices (128 rows each)."""
            sc = sc_ps.tile([P, 2, QB], f32, tag="sc", name="pv")
            for i in range(QS):
                nc.tensor.matmul(
                    sc[:, i // 2, (i % 2) * E:(i % 2 + 1) * E],
                    lhsT=src4[i],
                    rhs=wv8[:, :, :],
                    start=True, stop=True, perf_mode=DR,
                )
            nc.vector.tensor_add(
                out=v_sb[:, dst_c * QS:(dst_c + 1) * QS, :E],
                in0=sc[:].rearrange("p a (b e) -> p (a b) e", b=2),
                in1=_bc_mid(bv_bc[:], QS),
            )

        def proj_v2_sp(v_sb, wv8, bv_bc, dst_c, r0):
            """V proj from the interleaved transposed source.  Dual-fp8
            ldweights rejects byte-interleaved planes, so run one plain fp8
            matmul per plane (stride-2 stationary columns are legal)."""
            sc = sc_ps.tile([P, 2, QB], f32, tag="sc", name="pv2")
            for i in range(QS):
                for b in range(2):
                    nc.tensor.matmul(
                        sc[:, i // 2, (i % 2) * E:(i % 2 + 1) * E],
                        lhsT=oT_sb[:, r0 + i * P:r0 + (i + 1) * P, b],
                        rhs=wv8[:, b, :],
                        start=(b == 0), stop=(b == 1),
                    )
            nc.vector.tensor_add(
                out=v_sb[:, dst_c * QS:(dst_c + 1) * QS, :E],
                in0=sc[:].rearrange("p a (b e) -> p (a b) e", b=2),
                in1=_bc_mid(bv_bc[:], QS),
            )

        def attention_block(kT_sb, v_sb, qT_sb, qb, resid_tiles, g_bc, be_bc,
                            out_cb, pair_order=None, act_rsqrt=False,
                            mid_emit=None, out_dst=None):
            """One 512-query attention block + residual + layernorm.

            Scores for a key-chunk pair land in one 2-bank PSUM group, get
            exp'd by a single fp8-out ACT instruction, and feed DoubleRow
            att@V matmuls (contraction = the pair's 256 keys).  Scores for
            pair j+1 are emitted before pair j's att@V so the Tensor engine
            never idles behind the ACT dependency.

            The layernorm rstd is computed with a DVE-only Newton rsqrt so
            the Scalar engine never leaves the Exp activation table.
            """
            if pair_order is None:
                pair_order = [2 * j for j in range(NP_)]
            po = [
                o_ps.tile([P, E + 1], f32, tag="ops", name=f"po{i}")
                for i in range(QS)
            ]
            resid = [f() for f in resid_tiles]
            mid_exp = None

            def emit_scores(kc0):
                sc = sc_ps.tile([P, 2, QB], f32, tag="sc", name="scores")
                for t in range(2):
                    nc.tensor.matmul(
                        sc[:, t, :],
                        lhsT=kT_sb[:, :, (kc0 + t) * P:(kc0 + t + 1) * P],
                        rhs=qT_sb[:, :, qb * QB:(qb + 1) * QB],
                        start=True, stop=True, perf_mode=DR,
                    )
                return sc

            sc_cur = emit_scores(pair_order[0])
            last_ex = None
            for j, kc0 in enumerate(pair_order):
                if mid_emit and j in mid_emit:
                    mid_emit[j](last_ex)
                ex = expp.tile([P, 2, QB], f8, tag="exp")
                last_ex = nc.scalar.activation(
                    out=ex[:], in_=sc_cur[:],
                    func=mybir.ActivationFunctionType.Exp,
                    scale=float(SCALE),
                )
                if j + 1 < NP_:
                    sc_cur = emit_scores(pair_order[j + 1])
                for qs in range(QS):
                    nc.tensor.matmul(
                        po[qs][:],
                        lhsT=ex[:, :, qs * P:(qs + 1) * P],
                        rhs=v_sb[:, kc0:kc0 + 2, :],
                        start=(j == 0),
                        stop=(j == NP_ - 1),
                        perf_mode=DR,
                    )
            # Epilogue.  LN is scale-invariant, so instead of dividing the
            # attention numerator by the softmax denominator d (= po[:,E]),
            # normalize w = d*x + po -- LN(w) == LN(po/d + x) exactly (d>0),
            # and the reciprocal disappears.  EPS (1e-5 vs var~1) is dropped;
            # it is ~5e-6 relative, far below the fp8 noise floor.
            ats = []
            mv4 = stats.tile([P, nc.vector.BN_AGGR_DIM, QS], f32, tag="mv4")
            for qs in range(QS):
                at = work.tile([P, E], f32, tag="attn", name=f"at{qs}")
                nc.vector.tensor_scalar_mul(
                    out=at[:], in0=resid[qs], scalar1=po[qs][:, E:E + 1]
                )
                nc.vector.tensor_add(out=at[:], in0=at[:], in1=po[qs][:, :E])
                st = stats.tile([P, nc.vector.BN_STATS_DIM], f32, tag="bst")
                nc.vector.bn_stats(out=st[:], in_=at[:])
                nc.vector.bn_aggr(out=mv4[:, :, qs], in_=st[:])
                ats.append(at)
            # rstd = rsqrt(var) via bitcast seed + 2 Newton steps (DVE-only,
            # magnitude-agnostic, so ACT never leaves the Exp table).
            v4 = mv4[:, 1, :]
            rstd = stats.tile([P, QS], f32, tag="rstd")
            ri = rstd[:].bitcast(mybir.dt.int32)
            nc.vector.tensor_scalar(
                out=ri, in0=v4.bitcast(mybir.dt.int32), scalar1=1, scalar2=0,
                op0=mybir.AluOpType.logical_shift_right,
                op1=mybir.AluOpType.bypass,
            )
            nc.vector.tensor_scalar(
                out=ri, in0=ri, scalar1=-1, scalar2=0x5F3759DF,
                op0=mybir.AluOpType.mult, op1=mybir.AluOpType.add,
            )
            tmp = stats.tile([P, QS], f32, tag="nwt")
            for _ in range(2):
                nc.vector.tensor_mul(out=tmp[:], in0=rstd[:], in1=rstd[:])
                nc.vector.tensor_mul(out=tmp[:], in0=tmp[:], in1=v4)
                nc.vector.tensor_scalar(
                    out=tmp[:], in0=tmp[:], scalar1=-0.5, scalar2=1.5,
                    op0=mybir.AluOpType.mult, op1=mybir.AluOpType.add,
                )
                nc.vector.tensor_mul(out=rstd[:], in0=rstd[:], in1=tmp[:])
            for qs in range(QS):
                at = ats[qs]
                nc.vector.tensor_scalar(
                    out=at[:], in0=at[:],
                    scalar1=mv4[:, 0, qs:qs + 1], scalar2=rstd[:, qs:qs + 1],
                    op0=mybir.AluOpType.subtract, op1=mybir.AluOpType.mult,
                )
                nc.vector.tensor_mul(out=at[:], in0=at[:], in1=g_bc[:])
                dst = out_dst(qs) if out_dst else at
                ins = nc.vector.tensor_add(out=dst[:], in0=at[:], in1=be_bc[:])
                if qs == 1:
                    mid_exp = ins
                out_cb(qs, dst)
            return mid_exp

        # ---------------- layer 1 ----------------
        xT_sb = srcT_pool.tile([P, DC, S], f8, tag="srcT")
        kT1 = kt_pool.tile([P, DC, S], f8, tag="kt")
        v1 = v_pool.tile([P, KC, E + 1], f8, tag="v")
        nc.vector.memset(v1[:, :, E:E + 1], 1.0)
        qT1 = qt_pool.tile([P, DC, SQ], f8, tag="qt")
        xT_r = xT.rearrange("p (dc s) -> p dc s", dc=DC)
        G = 1024

        def l1_chunk(c):
            if c % 2 == 0:
                g = c // 2
                nc.sync.dma_start(
                    out=xT_sb[:, :, g * G:(g + 1) * G],
                    in_=xT_r[:, :, g * G:(g + 1) * G],
                )
            proj_kq(kT1, w_sb["wkt1"], bias_sb["bk1"], c,
                    xT_sb[:, :, c * QB:(c + 1) * QB])
            proj_v(v1, w_sb["wvt1"], bcast_sb["bv1"], c,
                   [xT_sb[:, :, (c * QS + i) * P:(c * QS + i + 1) * P]
                    for i in range(QS)])
            if c < NQB:
                proj_kq(qT1, w_sb["wqt1"], bias_sb["bq1"], c,
                        xT_sb[:, :, c * QB:(c + 1) * QB])

        o_tiles = []

        def l2_parts(qb):
            """Layer-2 chunk work for block qb, split into closures spread
            across later blocks' mid_emit slots.  "load" pulls the two
            AllGathered transposed halves into the key source (contributor
            order == canonical order on every rank); the projections follow
            once the loads have landed."""
            def part_load(after):
                for half in range(2):
                    r0 = half * SQ + qb * QB
                    nc.sync.dma_start(
                        out=oT_u16[:, r0:r0 + QB],
                        in_=o_chunksT[qb][half * P:(half + 1) * P, :],
                    )

            def part_q(after):
                proj_kq(qT2, w_sb["wqt2"], bias_sb["bq2"], qb,
                        oqT_sb[:, qb * QB:(qb + 1) * QB, :].rearrange(
                            "p r b -> p b r"))

            def make_kv(half):
                def part_kv(after):
                    r0 = half * SQ + qb * QB
                    c = half * NQB + qb
                    proj_kq(kT2, w_sb["wkt2"], bias_sb["bk2"], c,
                            oT_sb[:, r0:r0 + QB, :].rearrange("p r b -> p b r"))
                    proj_v2_sp(v2, w_sb["wvt2"], bcast_sb["bv2"], c, r0)
                return part_kv

            return {"load": part_load, "q": part_q,
                    "kv0": make_kv(0), "kv1": make_kv(1)}

        def make_resid1(qb, qs):
            def f():
                t = work.tile([P, E], f32, tag="xq")
                nc.sync.dma_start(
                    out=t[:], in_=xq[(qb * QS + qs) * P:(qb * QS + qs + 1) * P, :]
                )
                return t
            return f

        # layer-2 source tiles: LN1 output transposed as packed fp8 pairs,
        # oT_sb[p, r, b] = o[r, 2p+b] (written chunk-by-chunk as AllGather
        # results land, interleaved with the remaining layer-1 attention)
        oT_sb = srcT_pool.tile([P, S, 2], f8, tag="oT")
        oT_u16 = oT_sb[:].bitcast(u16)
        oqT_sb = qt_pool.tile([P, SQ, 2], f8, tag="oqT")
        oqT_u16 = oqT_sb[:].bitcast(u16)
        kT2 = kt_pool.tile([P, DC, S], f8, tag="kt")
        v2 = v_pool.tile([P, KC, E + 1], f8, tag="v")
        nc.vector.memset(v2[:, :, E:E + 1], 1.0)
        qT2 = qt_pool.tile([P, DC, SQ], f8, tag="qt")

        chunk_parts = {qb: l2_parts(qb) for qb in range(NQB)}

        def dst1(qs):
            ot = o_pool.tile([P, E], f32, tag="okeep")
            o_tiles.append(ot)
            return ot

        def emit_exchange(qb):
            # transpose block qb's LN1 rows locally, bounce to DRAM, and
            # exchange the transposed chunk within the batch pair.
            nc.sync.dma_start_transpose(
                out=oqT_u16[:, qb * QB:(qb + 1) * QB],
                in_=o_bounce[qb * QB:(qb + 1) * QB, :].bitcast(u16),
            )
            nc.sync.dma_start(
                out=oT_bounce[qb][:, :],
                in_=oqT_u16[:, qb * QB:(qb + 1) * QB],
            )
            nc.gpsimd.collective_compute(
                "AllGather",
                mybir.AluOpType.bypass,
                ins=[oT_bounce[qb][:, :].opt()],
                outs=[o_chunksT[qb].opt()],
                replica_groups=[[0, 1], [2, 3], [4, 5], [6, 7]],
            )

        # qb0's attention is interleaved with the projection loop itself:
        # chunk c lands just before the score pair that first needs it.
        l1_chunk(0)
        _load_late_consts()
        for qb in range(NQB):
            def out1(qs, ot, qb=qb):
                ob = work.tile([P, E], f8, tag="obf")
                nc.vector.tensor_copy(out=ob[:], in_=ot[:])
                r = (qb * QS + qs) * P
                nc.sync.dma_start(out=o_bounce[r:r + P, :], in_=ob[:])

            mid = {}
            if qb == 0:
                mid = {2 * c - 1: (lambda a, c=c: l1_chunk(c))
                       for c in range(1, S // QB)}
            if qb >= 2:
                # layer-2 chunk work for block qb-2 (its exchange ran during
                # block qb-1; the loads were queued at qb-1's tail)
                parts = chunk_parts[qb - 2]
                mid[3] = (lambda a, p=parts["q"]: p(a))
                mid[5] = (lambda a, p=parts["kv0"]: p(a))
                mid[7] = (lambda a, p=parts["kv1"]: p(a))
            if qb >= 1:
                mid[15] = (lambda a, p=chunk_parts[qb - 1]["load"]: p(a))

            attention_block(
                kT1, v1, qT1, qb,
                [make_resid1(qb, qs) for qs in range(QS)],
                bcast_sb["g1"], bcast_sb["be1"], out1, out_dst=dst1,
                mid_emit=mid,
            )
            emit_exchange(qb)

        # key-chunk pairs ordered by AllGather-chunk arrival:
        # chunk cc covers kcs [4cc, 4cc+4); arrival order 0,4,1,5,2,6,3,7
        pair_order2 = [
            cc * QS + 2 * h
            for cc in [0, 4, 1, 5, 2, 6, 3, 7]
            for h in range(2)
        ]
        for qb in range(NQB):
            def out2(qs, at, qb=qb):
                r = (qb * QS + qs) * P
                nc.sync.dma_start(out=y[r:r + P, :], in_=at[:])

            if qb == 0:
                p2, p3 = chunk_parts[2], chunk_parts[3]
                mid = {3: (lambda a: p2["q"](a)),
                       5: (lambda a: p2["kv0"](a)),
                       7: (lambda a: p2["kv1"](a)),
                       9: (lambda a: p3["load"](a)),
                       11: (lambda a: p3["kv0"](a)),
                       13: (lambda a: p3["kv1"](a))}
            elif qb == 1:
                mid = {3: (lambda a: chunk_parts[3]["q"](a))}
            else:
                mid = None

            attention_block(
                kT2, v2, qT2, qb,
                [
                    (lambda qs=qs, qb=qb: o_tiles[qb * QS + qs])
                    for qs in range(QS)
                ],
                bcast_sb["g2"], bcast_sb["be2"], out2,
                pair_order=pair_order2,
                mid_emit=mid,
            )


def _prep_inputs(x, Wq1, bq1, Wk1, bk1, Wv1, bv1, Wq2, bq2, Wk2, bk2, Wv2,
                 bv2, g1, beta1, g2, beta2):
    f8l = ml_dtypes.float8_e4m3
    shared = {}

    def _stripe(a2d):
        # block stripe: [p, dc*n + j] = a2d[dc*128 + p, j]
        e_in, n = a2d.shape
        return np.ascontiguousarray(
            a2d.reshape(DC, P, n).transpose(1, 0, 2).reshape(P, DC * n)
        ).astype(f8l)

    def _stripe_il(a2d):
        # interleaved stripe: [p, b*n + j] = a2d[2p + b, j]
        e_in, n = a2d.shape
        return np.ascontiguousarray(
            a2d.reshape(P, 2, n).reshape(P, 2 * n)
        ).astype(f8l)

    for n, w in [("wqt1", Wq1), ("wkt1", Wk1), ("wvt1", Wv1)]:
        shared[n] = _stripe(np.asarray(w, np.float32).T)
    for n, w in [("wqt2", Wq2), ("wkt2", Wk2), ("wvt2", Wv2)]:
        shared[n] = _stripe_il(np.asarray(w, np.float32).T)
    for n, v in [("bq1", bq1), ("bk1", bk1), ("bv1", bv1),
                 ("bq2", bq2), ("bk2", bk2), ("bv2", bv2),
                 ("g1", g1), ("be1", beta1), ("g2", g2), ("be2", beta2)]:
        shared[n] = np.ascontiguousarray(np.asarray(v, np.float32))

    x = np.asarray(x, np.float32)
    in_maps = []
    for c in range(N_CORES):
        b, h = c // 2, c % 2
        xb = x[b]
        if h:
            xb = np.concatenate([xb[SQ:], xb[:SQ]], axis=0)
        m = dict(shared)
        m["xT"] = _stripe(np.ascontiguousarray(xb.T))
        m["xq"] = np.ascontiguousarray(xb[:SQ])
        in_maps.append(m)
    return in_maps


def _get_compiled():
    global _COMPILED
    if _COMPILED is None:
        _COMPILED = _build()
    return _COMPILED


def run(trace=False, **inputs):
    nc = _get_compiled()
    in_maps = _prep_inputs(**inputs)
    last_err = None
    for _ in range(3):
        try:
            res = bass_utils.run_bass_kernel_spmd(
                nc, in_maps, core_ids=list(range(N_CORES)), trace=trace
            )
            break
        except Exception as e:  # transient NRT device errors; retry
            last_err = e
    else:
        raise last_err
    out = np.empty((B, S, E), np.float32)
    for c in range(N_CORES):
        b, h = c // 2, c % 2
        out[b, h * SQ:(h + 1) * SQ] = res.results[c]["y"]
    return out, res


def kernel(**inputs):
    out, _ = run(trace=False, **inputs)
    return out


# revision 27
# speedup vs baseline: 1.0178x; 1.0178x over previous
"""Two-block single-head transformer (B=4, S=4096, E=256) on 8 TRN2 NeuronCores.

Sharding: core c -> batch b=c//2, query-half h=c%2 (2048 query rows each).
Each core receives its batch's x ROLLED so that its own query rows are always
rows [0:2048] -- this keeps the on-device program identical across cores
(pure SPMD, no partition-id branching).  Attention is permutation-invariant
over keys, so layer-1 may use the rolled key order.  Layer-2 keys come from a
pairwise AllGather of the LN1 outputs in canonical order.

Math per layer (matches torch reference):
  q/k/v = x @ W.T + b ; att = softmax((q k^T)/sqrt(S)) ; o = att v
  layernorm over E with gamma/beta.

Datapath: every matmul runs in fp8-e4m3 with MatmulPerfMode.DoubleRow, which
contracts two 128-row K-subtiles per pass (2x bf16 FLOP rate).  Q^T/K^T
tiles are laid out [128, 2, n] with dim1 = the two halves of E, exp'd score
tiles [128, 2, 512] with dim1 = a pair of 128-key chunks -- both exactly the
DoubleRow operand shape.  Scores for a key-chunk PAIR accumulate into one
2-bank PSUM group and are exp'd by a single ACT instruction (amortizing the
~352-cycle ACT fixed overhead), output straight to fp8 for the att@V matmul.
A ones-column appended to V makes the softmax denominator fall out of the
same accumulation.

The scores->exp->attV chain is software-pipelined: scores for pair j+1 are
emitted BEFORE the att@V matmuls of pair j, so the in-order Tensor stream
never sits behind an ACT wait (period = max(tensor, act) per pair instead of
their sum).

Layer-2 projections read the LN1 output transposed.  The transpose DMA only
supports 2-byte dtypes, so the fp8 LN1 output is transposed as packed uint16
PAIRS (bitcast), giving oT[p, r, b] = o[r, 2p+b]; the layer-2 weights are
host-striped with the matching interleaved E-pairing so the DoubleRow
contraction (dim1 = b) still sums over all of E.

PSUM budget: 4 banks att@V accumulators + 2x2-bank double-buffered score
groups = 8.  Projection matmuls borrow score-group tiles from the same pool.

The layer boundary is pipelined: the AllGather is split into one chunk per
512-query block, and the layer-2 transposes/projections are chunked behind
those, so they overlap the remaining layer-1 attention instead of
serializing after it.
"""

import sys

sys.path.insert(0, "/opt/trn_rl_repo")

import numpy as np
import ml_dtypes

import concourse.bass as bass
import concourse.tile as tile
from concourse import bacc, mybir
from concourse import bass_utils

f32 = mybir.dt.float32
bf16 = mybir.dt.bfloat16
f8 = mybir.dt.float8e4
u16 = mybir.dt.uint16
DR = mybir.MatmulPerfMode.DoubleRow

B, S, E = 4, 4096, 256
P = 128
SQ = S // 2          # query rows per core
DC = E // P          # 2 chunks of the head dim
KC = S // P          # 32 key chunks
NP_ = KC // 2        # 16 key-chunk pairs
QB = 512             # query block (matmul moving dim)
NQB = SQ // QB       # 4 query blocks per core
QS = QB // P         # 4 query sub-blocks per block
N_CORES = 8
EPS = 1e-5
SCALE = 1.0 / np.sqrt(np.float32(S))

_COMPILED = None


def _broadcast_ap(vec_ap, parts, n):
    """[n] DRAM vector -> [parts, n] partition-broadcast access pattern."""
    return bass.AP(
        tensor=vec_ap.tensor,
        offset=vec_ap.offset,
        ap=[[0, parts], [1, n]],
    )


def _bc_mid(ap2d, reps):
    """[P, n] SBUF AP -> [P, reps, n] with a stride-0 middle dim."""
    return bass.AP(
        tensor=ap2d.tensor,
        offset=ap2d.offset,
        ap=[list(ap2d.ap[0]), [0, reps], list(ap2d.ap[1])],
    )


def _build():
    nc = bacc.Bacc(
        "TRN2", target_bir_lowering=False, debug=False, num_devices=N_CORES
    )

    # --- kernel I/O (per core) ---
    xT = nc.dram_tensor("xT", [P, DC * S], f8, kind="ExternalInput").ap()
    xq = nc.dram_tensor("xq", [SQ, E], f32, kind="ExternalInput").ap()
    wts = {
        n: nc.dram_tensor(n, [P, DC * E], f8, kind="ExternalInput").ap()
        for n in ["wqt1", "wkt1", "wvt1", "wqt2", "wkt2", "wvt2"]
    }
    vecs = {
        n: nc.dram_tensor(n, [E], f32, kind="ExternalInput").ap()
        for n in ["bq1", "bk1", "bv1", "bq2", "bk2", "bv2",
                  "g1", "be1", "g2", "be2"]
    }
    y = nc.dram_tensor("y", [SQ, E], f32, kind="ExternalOutput").ap()

    with tile.TileContext(nc) as tc:
        _emit(nc, tc, xT, xq, wts, vecs, y)

    nc.compile()
    return nc


def _emit(nc, tc, xT, xq, wts, vecs, y):
    from contextlib import ExitStack

    ctx = ExitStack()
    with ctx:
        const = ctx.enter_context(tc.tile_pool(name="const", bufs=1))
        srcT_pool = ctx.enter_context(tc.tile_pool(name="srcT", bufs=1))
        kt_pool = ctx.enter_context(tc.tile_pool(name="kt", bufs=2))
        v_pool = ctx.enter_context(tc.tile_pool(name="v", bufs=2))
        qt_pool = ctx.enter_context(tc.tile_pool(name="qt", bufs=2))
        o_pool = ctx.enter_context(tc.tile_pool(name="okeep", bufs=16))
        work = ctx.enter_context(tc.tile_pool(name="work", bufs=4))
        expp = ctx.enter_context(tc.tile_pool(name="expp", bufs=4))
        stats = ctx.enter_context(tc.tile_pool(name="stats", bufs=8))
        dram = ctx.enter_context(tc.tile_pool(name="dram", bufs=1, space="DRAM"))
        # score-pair groups (2 banks each, double-buffered); projection
        # matmuls borrow tiles from the same pool
        sc_ps = ctx.enter_context(tc.tile_pool(name="sc_ps", bufs=2, space="PSUM"))
        o_ps = ctx.enter_context(tc.tile_pool(name="o_ps", bufs=4, space="PSUM"))

        # --- constants (sync HWDGE queue, in order of first use) ---
        w_sb, bias_sb, bcast_sb = {}, {}, {}

        def _load_w(n):
            t = const.tile([P, DC, E], f8, tag=f"w_{n}", name=f"w_{n}")
            nc.sync.dma_start(
                out=t[:], in_=wts[n].rearrange("p (dc o) -> p dc o", dc=DC)
            )
            w_sb[n] = t

        def _load_b(n):
            t = const.tile([P, DC], f32, tag=f"b_{n}", name=f"b_{n}")
            nc.sync.dma_start(
                out=t[:], in_=vecs[n].rearrange("(dc p) -> p dc", p=P)
            )
            bias_sb[n] = t

        def _load_bc(n):
            t = const.tile([P, E], f32, tag=f"bc_{n}", name=f"bc_{n}")
            nc.sync.dma_start(out=t[:], in_=_broadcast_ap(vecs[n], P, E))
            bcast_sb[n] = t

        for n in ["wkt1", "wvt1", "wqt1"]:
            _load_w(n)
        _load_b("bk1")
        _load_b("bq1")
        _load_bc("bv1")

        def _load_late_consts():
            for n in ["g1", "be1"]:
                _load_bc(n)
            for n in ["wqt2", "wkt2", "wvt2"]:
                _load_w(n)
            for n in ["bq2", "bk2"]:
                _load_b(n)
            for n in ["bv2", "g2", "be2"]:
                _load_bc(n)

        o_bounce = dram.tile([SQ, E], f8)
        # the layer-boundary exchange ships TRANSPOSED chunks: each block's
        # LN1 rows are transposed locally (oqT), bounced to DRAM, and
        # AllGathered as [128, QB] uint16 tiles -- so the layer-2 key source
        # needs only plain contiguous loads after the collective, never a
        # transpose gated on remote data.
        oT_bounce = [
            dram.tile([P, QB], u16, name=f"otb{i}") for i in range(NQB)
        ]
        o_chunksT = [
            dram.tile([2 * P, QB], u16, name=f"agchunk{i}") for i in range(NQB)
        ]

        # ---- fp8 DoubleRow projections.  src is [P, 2, n] with dim1 the
        # contraction-pair index (E-halves for layer 1, E-pairs for the
        # transposed layer-2 source); w must be striped to match. ----
        def proj_kq(dstT, w8, bk, dst_c, src):
            sc = sc_ps.tile([P, 2, QB], f32, tag="sc", name="pkq")
            for oc in range(DC):
                nc.tensor.matmul(
                    sc[:, oc, :],
                    lhsT=w8[:, :, oc * P:(oc + 1) * P],
                    rhs=src,
                    start=True, stop=True, perf_mode=DR,
                )
            for oc in range(DC):
                nc.vector.tensor_scalar_add(
                    out=dstT[:, oc, dst_c * QB:(dst_c + 1) * QB],
                    in0=sc[:, oc, :],
                    scalar1=bk[:, oc:oc + 1],
                )

        def proj_v(v_sb, wv8, bv_bc, dst_c, src4):
            """src4: list of 4 [P, 2, P] stationary slices (128 rows each)."""
            sc = sc_ps.tile([P, 2, QB], f32, tag="sc", name="pv")
            for i in range(QS):
                nc.tensor.matmul(
                    sc[:, i // 2, (i % 2) * E:(i % 2 + 1) * E],
                    lhsT=src4[i],
                    rhs=wv8[:, :, :],
                    start=True, stop=True, perf_mode=DR,
                )
            nc.vector.tensor_add(
                out=v_sb[:, dst_c * QS:(dst_c + 1) * QS, :E],
                in0=sc[:].rearrange("p a (b e) -> p (a b) e", b=2),
                in1=_bc_mid(bv_bc[:], QS),
            )

        def proj_v2_sp(v_sb, wv8, bv_bc, dst_c, r0):
            """V proj from the interleaved transposed source.  Dual-fp8
            ldweights rejects byte-interleaved planes, so run one plain fp8
            matmul per plane (stride-2 stationary columns are legal)."""
            sc = sc_ps.tile([P, 2, QB], f32, tag="sc", name="pv2")
            for i in range(QS):
                for b in range(2):
                    nc.tensor.matmul(
                        sc[:, i // 2, (i % 2) * E:(i % 2 + 1) * E],
                        lhsT=oT_sb[:, r0 + i * P:r0 + (i + 1) * P, b],
                        rhs=wv8[:, b, :],
                        start=(b == 0), stop=(b == 1),
                    )
            nc.vector.tensor_add(
                out=v_sb[:, dst_c * QS:(dst_c + 1) * QS, :E],
                in0=sc[:].rearrange("p a (b e) -> p (a b) e", b=2),
                in1=_bc_mid(bv_bc[:], QS),
            )

        def attention_block(kT_sb, v_sb, qT_sb, qb, resid_tiles, g_bc, be_bc,
                            out_cb, pair_order=None, act_rsqrt=False,
                            mid_emit=None, out_dst=None):
            """One 512-query attention block + residual + layernorm.

            Scores for a key-chunk pair land in one 2-bank PSUM group, get
            exp'd by a single fp8-out ACT instruction, and feed DoubleRow
            att@V matmuls (contraction = the pair's 256 keys).  Scores for
            pair j+1 are emitted before pair j's att@V so the Tensor engine
            never idles behind the ACT dependency.

            The layernorm rstd is computed with a DVE-only Newton rsqrt so
            the Scalar engine never leaves the Exp activation table.
            """
            if pair_order is None:
                pair_order = [2 * j for j in range(NP_)]
            po = [
                o_ps.tile([P, E + 1], f32, tag="ops", name=f"po{i}")
                for i in range(QS)
            ]
            resid = [f() for f in resid_tiles]
            mid_exp = None

            def emit_scores(kc0):
                sc = sc_ps.tile([P, 2, QB], f32, tag="sc", name="scores")
                for t in range(2):
                    nc.tensor.matmul(
                        sc[:, t, :],
                        lhsT=kT_sb[:, :, (kc0 + t) * P:(kc0 + t + 1) * P],
                        rhs=qT_sb[:, :, qb * QB:(qb + 1) * QB],
                        start=True, stop=True, perf_mode=DR,
                    )
                return sc

            sc_cur = emit_scores(pair_order[0])
            last_ex = None
            for j, kc0 in enumerate(pair_order):
                if mid_emit and j in mid_emit:
                    mid_emit[j](last_ex)
                ex = expp.tile([P, 2, QB], f8, tag="exp")
                last_ex = nc.scalar.activation(
                    out=ex[:], in_=sc_cur[:],
                    func=mybir.ActivationFunctionType.Exp,
                    scale=float(SCALE),
                )
                if j + 1 < NP_:
                    sc_cur = emit_scores(pair_order[j + 1])
                for qs in range(QS):
                    nc.tensor.matmul(
                        po[qs][:],
                        lhsT=ex[:, :, qs * P:(qs + 1) * P],
                        rhs=v_sb[:, kc0:kc0 + 2, :],
                        start=(j == 0),
                        stop=(j == NP_ - 1),
                        perf_mode=DR,
                    )
            # Epilogue.  LN is scale-invariant, so instead of dividing the
            # attention numerator by the softmax denominator d (= po[:,E]),
            # normalize w = d*x + po -- LN(w) == LN(po/d + x) exactly (d>0),
            # and the reciprocal disappears.  EPS (1e-5 vs var~1) is dropped;
            # it is ~5e-6 relative, far below the fp8 noise floor.
            ats = []
            mv4 = stats.tile([P, nc.vector.BN_AGGR_DIM, QS], f32, tag="mv4")
            for qs in range(QS):
                at = work.tile([P, E], f32, tag="attn", name=f"at{qs}")
                nc.vector.tensor_scalar_mul(
                    out=at[:], in0=resid[qs], scalar1=po[qs][:, E:E + 1]
                )
                nc.vector.tensor_add(out=at[:], in0=at[:], in1=po[qs][:, :E])
                st = stats.tile([P, nc.vector.BN_STATS_DIM], f32, tag="bst")
                nc.vector.bn_stats(out=st[:], in_=at[:])
                nc.vector.bn_aggr(out=mv4[:, :, qs], in_=st[:])
                ats.append(at)
            # rstd = rsqrt(var) via bitcast seed + 2 Newton steps (DVE-only,
            # magnitude-agnostic, so ACT never leaves the Exp table).
            v4 = mv4[:, 1, :]
            rstd = stats.tile([P, QS], f32, tag="rstd")
            ri = rstd[:].bitcast(mybir.dt.int32)
            nc.vector.tensor_scalar(
                out=ri, in0=v4.bitcast(mybir.dt.int32), scalar1=1, scalar2=0,
                op0=mybir.AluOpType.logical_shift_right,
                op1=mybir.AluOpType.bypass,
            )
            nc.vector.tensor_scalar(
                out=ri, in0=ri, scalar1=-1, scalar2=0x5F3759DF,
                op0=mybir.AluOpType.mult, op1=mybir.AluOpType.add,
            )
            tmp = stats.tile([P, QS], f32, tag="nwt")
            for _ in range(2):
                nc.vector.tensor_mul(out=tmp[:], in0=rstd[:], in1=rstd[:])
                nc.vector.tensor_mul(out=tmp[:], in0=tmp[:], in1=v4)
                nc.vector.tensor_scalar(
                    out=tmp[:], in0=tmp[:], scalar1=-0.5, scalar2=1.5,
                    op0=mybir.AluOpType.mult, op1=mybir.AluOpType.add,
                )
                nc.vector.tensor_mul(out=rstd[:], in0=rstd[:], in1=tmp[:])
            for qs in range(QS):
                at = ats[qs]
                nc.vector.tensor_scalar(
                    out=at[:], in0=at[:],
                    scalar1=mv4[:, 0, qs:qs + 1], scalar2=rstd[:, qs:qs + 1],
                    op0=mybir.AluOpType.subtract, op1=mybir.AluOpType.mult,
                )
                nc.vector.tensor_mul(out=at[:], in0=at[:], in1=g_bc[:])
                dst = out_dst(qs) if out_dst else at
                ins = nc.vector.tensor_add(out=dst[:], in0=at[:], in1=be_bc[:])
                if qs == 1:
                    mid_exp = ins
                out_cb(qs, dst)
            return mid_exp

        # ---------------- layer 1 ----------------
        xT_sb = srcT_pool.tile([P, DC, S], f8, tag="srcT")
        kT1 = kt_pool.tile([P, DC, S], f8, tag="kt")
        v1 = v_pool.tile([P, KC, E + 1], f8, tag="v")
        nc.vector.memset(v1[:, :, E:E + 1], 1.0)
        qT1 = qt_pool.tile([P, DC, SQ], f8, tag="qt")
        xT_r = xT.rearrange("p (dc s) -> p dc s", dc=DC)
        G = 1024

        def l1_chunk(c):
            nc.sync.dma_start(
                out=xT_sb[:, :, c * QB:(c + 1) * QB],
                in_=xT_r[:, :, c * QB:(c + 1) * QB],
            )
            proj_kq(kT1, w_sb["wkt1"], bias_sb["bk1"], c,
                    xT_sb[:, :, c * QB:(c + 1) * QB])
            proj_v(v1, w_sb["wvt1"], bcast_sb["bv1"], c,
                   [xT_sb[:, :, (c * QS + i) * P:(c * QS + i + 1) * P]
                    for i in range(QS)])
            if c < NQB:
                proj_kq(qT1, w_sb["wqt1"], bias_sb["bq1"], c,
                        xT_sb[:, :, c * QB:(c + 1) * QB])

        o_tiles = []

        def l2_parts(qb):
            """Layer-2 chunk work for block qb, split into closures spread
            across later blocks' mid_emit slots.  "load" pulls the two
            AllGathered transposed halves into the key source (contributor
            order == canonical order on every rank); the projections follow
            once the loads have landed."""
            def part_load(after):
                for half in range(2):
                    r0 = half * SQ + qb * QB
                    nc.sync.dma_start(
                        out=oT_u16[:, r0:r0 + QB],
                        in_=o_chunksT[qb][half * P:(half + 1) * P, :],
                    )

            def part_q(after):
                proj_kq(qT2, w_sb["wqt2"], bias_sb["bq2"], qb,
                        oqT_sb[:, qb * QB:(qb + 1) * QB, :].rearrange(
                            "p r b -> p b r"))

            def make_kv(half):
                def part_kv(after):
                    r0 = half * SQ + qb * QB
                    c = half * NQB + qb
                    proj_kq(kT2, w_sb["wkt2"], bias_sb["bk2"], c,
                            oT_sb[:, r0:r0 + QB, :].rearrange("p r b -> p b r"))
                    proj_v2_sp(v2, w_sb["wvt2"], bcast_sb["bv2"], c, r0)
                return part_kv

            return {"load": part_load, "q": part_q,
                    "kv0": make_kv(0), "kv1": make_kv(1)}

        def make_resid1(qb, qs):
            def f():
                t = work.tile([P, E], f32, tag="xq")
                nc.sync.dma_start(
                    out=t[:], in_=xq[(qb * QS + qs) * P:(qb * QS + qs + 1) * P, :]
                )
                return t
            return f

        # layer-2 source tiles: LN1 output transposed as packed fp8 pairs,
        # oT_sb[p, r, b] = o[r, 2p+b] (written chunk-by-chunk as AllGather
        # results land, interleaved with the remaining layer-1 attention)
        oT_sb = srcT_pool.tile([P, S, 2], f8, tag="oT")
        oT_u16 = oT_sb[:].bitcast(u16)
        oqT_sb = qt_pool.tile([P, SQ, 2], f8, tag="oqT")
        oqT_u16 = oqT_sb[:].bitcast(u16)
        kT2 = kt_pool.tile([P, DC, S], f8, tag="kt")
        v2 = v_pool.tile([P, KC, E + 1], f8, tag="v")
        nc.vector.memset(v2[:, :, E:E + 1], 1.0)
        qT2 = qt_pool.tile([P, DC, SQ], f8, tag="qt")

        chunk_parts = {qb: l2_parts(qb) for qb in range(NQB)}

        def dst1(qs):
            ot = o_pool.tile([P, E], f32, tag="okeep")
            o_tiles.append(ot)
            return ot

        def emit_exchange(qb):
            # transpose block qb's LN1 rows locally, bounce to DRAM, and
            # exchange the transposed chunk within the batch pair.
            nc.sync.dma_start_transpose(
                out=oqT_u16[:, qb * QB:(qb + 1) * QB],
                in_=o_bounce[qb * QB:(qb + 1) * QB, :].bitcast(u16),
            )
            nc.sync.dma_start(
                out=oT_bounce[qb][:, :],
                in_=oqT_u16[:, qb * QB:(qb + 1) * QB],
            )
            nc.gpsimd.collective_compute(
                "AllGather",
                mybir.AluOpType.bypass,
                ins=[oT_bounce[qb][:, :].opt()],
                outs=[o_chunksT[qb].opt()],
                replica_groups=[[0, 1], [2, 3], [4, 5], [6, 7]],
            )

        # qb0's attention is interleaved with the projection loop itself:
        # chunk c lands just before the score pair that first needs it.
        l1_chunk(0)
        _load_late_consts()
        for qb in range(NQB):
            def out1(qs, ot, qb=qb):
                ob = work.tile([P, E], f8, tag="obf")
                nc.vector.tensor_copy(out=ob[:], in_=ot[:])
                r = (qb * QS + qs) * P
                nc.sync.dma_start(out=o_bounce[r:r + P, :], in_=ob[:])

            mid = {}
            if qb == 0:
                mid = {2 * c - 1: (lambda a, c=c: l1_chunk(c))
                       for c in range(1, S // QB)}
            if qb == NQB - 1:
                # layer-2 projections for chunks 0/1 -- their exchanged data
                # has been resident since mid-qb2
                mid[3] = (lambda a: chunk_parts[0]["kv0"](a))
                mid[5] = (lambda a: chunk_parts[0]["kv1"](a))
                mid[7] = (lambda a: chunk_parts[1]["kv0"](a))
                mid[9] = (lambda a: chunk_parts[1]["kv1"](a))
                mid[11] = (lambda a: chunk_parts[0]["q"](a))
            if qb >= 1:
                mid[15] = (lambda a, p=chunk_parts[qb - 1]["load"]: p(a))

            attention_block(
                kT1, v1, qT1, qb,
                [make_resid1(qb, qs) for qs in range(QS)],
                bcast_sb["g1"], bcast_sb["be1"], out1, out_dst=dst1,
                mid_emit=mid,
            )
            emit_exchange(qb)

        # key-chunk pairs ordered by AllGather-chunk arrival:
        # chunk cc covers kcs [4cc, 4cc+4); arrival order 0,4,1,5,2,6,3,7
        pair_order2 = [
            cc * QS + 2 * h
            for cc in [0, 4, 1, 5, 2, 6, 3, 7]
            for h in range(2)
        ]
        for qb in range(NQB):
            def out2(qs, at, qb=qb):
                r = (qb * QS + qs) * P
                nc.sync.dma_start(out=y[r:r + P, :], in_=at[:])

            if qb == 0:
                p2, p3 = chunk_parts[2], chunk_parts[3]
                mid = {3: (lambda a: p2["kv0"](a)),
                       5: (lambda a: p2["kv1"](a)),
                       9: (lambda a: p3["load"](a)),
                       11: (lambda a: p3["kv0"](a)),
                       13: (lambda a: p3["kv1"](a)),
                       14: (lambda a: chunk_parts[1]["q"](a))}
            elif qb == 1:
                mid = {3: (lambda a: chunk_parts[2]["q"](a))}
            elif qb == 2:
                mid = {3: (lambda a: chunk_parts[3]["q"](a))}
            else:
                mid = None

            attention_block(
                kT2, v2, qT2, qb,
                [
                    (lambda qs=qs, qb=qb: o_tiles[qb * QS + qs])
                    for qs in range(QS)
                ],
                bcast_sb["g2"], bcast_sb["be2"], out2,
                pair_order=pair_order2,
                mid_emit=mid,
            )


def _prep_inputs(x, Wq1, bq1, Wk1, bk1, Wv1, bv1, Wq2, bq2, Wk2, bk2, Wv2,
                 bv2, g1, beta1, g2, beta2):
    f8l = ml_dtypes.float8_e4m3
    shared = {}

    def _stripe(a2d):
        # block stripe: [p, dc*n + j] = a2d[dc*128 + p, j]
        e_in, n = a2d.shape
        return np.ascontiguousarray(
            a2d.reshape(DC, P, n).transpose(1, 0, 2).reshape(P, DC * n)
        ).astype(f8l)

    def _stripe_il(a2d):
        # interleaved stripe: [p, b*n + j] = a2d[2p + b, j]
        e_in, n = a2d.shape
        return np.ascontiguousarray(
            a2d.reshape(P, 2, n).reshape(P, 2 * n)
        ).astype(f8l)

    for n, w in [("wqt1", Wq1), ("wkt1", Wk1), ("wvt1", Wv1)]:
        shared[n] = _stripe(np.asarray(w, np.float32).T)
    for n, w in [("wqt2", Wq2), ("wkt2", Wk2), ("wvt2", Wv2)]:
        shared[n] = _stripe_il(np.asarray(w, np.float32).T)
    for n, v in [("bq1", bq1), ("bk1", bk1), ("bv1", bv1),
                 ("bq2", bq2), ("bk2", bk2), ("bv2", bv2),
                 ("g1", g1), ("be1", beta1), ("g2", g2), ("be2", beta2)]:
        shared[n] = np.ascontiguousarray(np.asarray(v, np.float32))

    x = np.asarray(x, np.float32)
    in_maps = []
    for c in range(N_CORES):
        b, h = c // 2, c % 2
        xb = x[b]
        if h:
            xb = np.concatenate([xb[SQ:], xb[:SQ]], axis=0)
        m = dict(shared)
        m["xT"] = _stripe(np.ascontiguousarray(xb.T))
        m["xq"] = np.ascontiguousarray(xb[:SQ])
        in_maps.append(m)
    return in_maps


def _get_compiled():
    global _COMPILED
    if _COMPILED is None:
        _COMPILED = _build()
    return _COMPILED


def run(trace=False, **inputs):
    nc = _get_compiled()
    in_maps = _prep_inputs(**inputs)
    last_err = None
    for _ in range(3):
        try:
            res = bass_utils.run_bass_kernel_spmd(
                nc, in_maps, core_ids=list(range(N_CORES)), trace=trace
            )
            break
        except Exception as e:  # transient NRT device errors; retry
            last_err = e
    else:
        raise last_err
    out = np.empty((B, S, E), np.float32)
    for c in range(N_CORES):
        b, h = c // 2, c % 2
        out[b, h * SQ:(h + 1) * SQ] = res.results[c]["y"]
    return out, res


def kernel(**inputs):
    out, _ = run(trace=False, **inputs)
    return out
